# revision 12
# baseline (speedup 1.0000x reference)
"""Single-head GAT (DGL GATConv) forward on 8 Trainium2 NeuronCores.

Strategy (graph/data parallel, per the sharding hint) — prefix-sum
aggregation. This environment executes instructions at a large flat cost
(~40-200us each, engine-dependent: DVE ~60us, Act ~110us, PE matmul
~190us, ap_gather [128,4096,4] ~120us, AllGather ~1ms), so the kernel is
shaped to minimize INSTRUCTION COUNT, using the largest possible access
patterns per instruction (~150 productive instructions per iteration vs
~1270 for the indirect-DMA + one-hot-matmul formulation; 67.3ms -> 7.8ms).

  - Nodes padded 10000 -> 10240, degree-balance relabeled into 80 windows
    of 128 dst nodes such that each window has <= 4096 in-edges; 10
    windows per core (1280 dst nodes/core); edges dst-sorted per window.
  - Phase 1 (per core): hT = W^T @ feats^T computed FEATURE-MAJOR in 12
    matmuls (the minimum for 256k x 256f x 1280n at [128,512] PSUM
    quanta): gather-table row n holds bf16 (h[n,p], h[n,p+128],
    el-as-fp16-bits, pad) on partition p.  el/er are dotted out of the
    PSUM h directly with one DVE multiply + one gpsimd
    partition_all_reduce (replicated to all partitions), avoiding any
    extra PE work.  Staged to DRAM, AllGather -> table [128, 10240, 4].
  - Phase 2 (per core, per window): ONE ap_gather pulls all 4096 edges'
    (h-pair, el) columns; a second ap_gather pulls per-edge er (f32, by
    local dst). Logits e = lrelu(el+er) batched on [128,4096]; the
    per-window max is subtracted before exp (softmax shift-invariance per
    dst; all of a dst's edges live in one window, so it cancels exactly)
    which keeps the prefix sums O(1)-magnitude and the boundary
    differences accurate. w_e and wh[e] = w_e * h-pair are packed into a
    [128, 3, 4+4096] bf16 buffer (4 leading zeros per channel); ONE
    tensor_tensor_scan computes the running sum of all 3 channels; ONE
    ap_gather reads the 3*129 segment-boundary prefix values; ONE
    subtract turns them into per-dst (u0, u1, s) = (sum w*h_p,
    sum w*h_{p+128}, sum w).  The gather buffer, logit scratch, and scan
    output share one 49KB SBUF slot via bitcast views (lifetimes are
    disjoint; the tile framework orders them by buffer dependencies).
    NOTE: every ap_gather index slice must be 4-byte aligned (the comb
    record is padded to 544 int16) — odd element offsets make the gpsimd
    ucode mis-read whole 32-byte idx blocks.
  - Epilogue: out^T = u/s + bias in 3 DVE ops + 1 DMA; host de-transposes
    and un-relabels.

kernel(**inputs) takes full unsharded inputs, returns [10000, 256] fp32.
"""

import numpy as np
import ml_dtypes

N = 10000
E = 320000
D = 256
NPAD = 10240
NCORES = 8
SH = NPAD // NCORES          # 1280 nodes per core
WINN = 128                   # dst nodes per window
NW = SH // WINN              # 10 windows per core
EPW = 4096                   # padded edges per window
CW = 4100                    # scan channel width: 4 zero pads + 4096
NEG_SLOPE = 0.2
REPEAT = 1                   # whole-pipeline repeats (differential timing)

_BF16 = ml_dtypes.bfloat16

_prog_cache = {}


def _wrap16(arr):
    """[Q] -> [128, Q//16] int16 in ap_gather's wrapped-index layout:
    idx[16g + r, f] = arr[f*16 + r], replicated across the 8 groups g."""
    q = arr.shape[0]
    w16 = arr.reshape(q // 16, 16).T.astype(np.int16)     # [16, Q//16]
    return np.broadcast_to(w16[None], (8, 16, q // 16)).reshape(128, q // 16)


def _prep_inputs(feats, W, attn_l, attn_r, bias, src, dst):
    """Host-side sharding/index prep. Returns (in_maps, newid)."""
    feats_pad = np.zeros((NPAD, D), np.float32)
    feats_pad[:N] = feats

    # degree-balanced node relabeling: greedy-pack nodes into 80 windows of
    # 128 so every window has <= EPW in-edges
    import heapq
    nwin_g = NPAD // WINN                     # 80
    deg = np.bincount(dst, minlength=NPAD).astype(np.int64)
    norder = np.argsort(-deg, kind="stable")
    bin_edges = np.zeros(nwin_g, np.int64)
    bin_count = np.zeros(nwin_g, np.int64)
    newid = np.empty(NPAD, np.int64)
    heap = [(0, b) for b in range(nwin_g)]
    heapq.heapify(heap)
    for n in norder:
        while True:
            _, b = heapq.heappop(heap)
            if bin_count[b] < WINN:
                break
        newid[n] = b * WINN + bin_count[b]
        bin_count[b] += 1
        bin_edges[b] += deg[n]
        if bin_count[b] < WINN:
            heapq.heappush(heap, (bin_edges[b], b))
    assert bin_edges.max() <= EPW, (bin_edges.max(), EPW)
    inv = np.empty(NPAD, np.int64)
    inv[newid] = np.arange(NPAD)
    feats_pad = feats_pad[inv]                # row j of feats_pad = new id j

    n_src = newid[src.astype(np.int64)]
    n_dst = newid[dst.astype(np.int64)]
    order = np.argsort(n_dst, kind="stable")
    s_src = n_src[order]
    s_dst = n_dst[order]

    win = s_dst // WINN                       # global window id, 0..79
    counts = np.bincount(win, minlength=nwin_g)
    starts = np.concatenate([[0], np.cumsum(counts)])

    # per-window index tables: h-gather (global src), er-gather (local dst),
    # boundary-gather (3 channels x 129 prefix positions in the scan buffer)
    # 544 (not 537): keeps every idx slice 64-byte aligned in all
    # windows; odd-element offsets make the gpsimd idx reads garbage
    comb = np.zeros((nwin_g, 128, 544), np.int16)
    for g in range(nwin_g):
        a, b = starts[g], starts[g + 1]
        k = b - a
        hidx = np.zeros(EPW, np.int64)
        eidx = np.zeros(EPW, np.int64)
        hidx[:k] = s_src[a:b]
        core_base = (g // NW) * SH
        eidx[:k] = s_dst[a:b] - core_base
        slot = s_dst[a:b] - g * WINN          # 0..127, ascending
        cum = np.zeros(129, np.int64)         # cum[j] = #edges with slot < j
        cnt = np.bincount(slot, minlength=128)
        cum[1:] = np.cumsum(cnt)
        bidx = np.zeros(400, np.int64)
        for ch in range(3):
            bidx[132 * ch:132 * ch + 129] = CW * ch + 3 + cum
        comb[g, :, 0:256] = _wrap16(hidx)
        comb[g, :, 256:512] = _wrap16(eidx)
        comb[g, :, 512:537] = _wrap16(bidx)

    # weight tiles: Wt4[kp, kh, ft, fp] = W[kp+128kh, fp+128ft]
    w4 = W.astype(np.float32).reshape(2, 128, 2, 128)          # [kh,kp,ft,fp]
    Wt4 = np.ascontiguousarray(w4.transpose(1, 0, 2, 3))       # [kp,kh,ft,fp]
    # attn4[p, lr, ft] = (attn_l | attn_r)[p + 128*ft]
    attn4 = np.stack([attn_l.astype(np.float32).reshape(2, 128).T,
                      attn_r.astype(np.float32).reshape(2, 128).T],
                     axis=1)                                   # [p, lr, ft]
    bias2 = np.ascontiguousarray(
        bias.astype(np.float32).reshape(2, 128).T)             # [p, d]

    in_maps = []
    for c in range(NCORES):
        fsh = feats_pad[c * SH:(c + 1) * SH]                   # [SH, 256]
        ftT = np.ascontiguousarray(
            fsh.T.reshape(2, 128, SH).transpose(1, 0, 2))      # [kp,kh,n]
        in_maps.append({
            "ftT": ftT,
            "Wt4": Wt4,
            "attn4": attn4,
            "bias2": bias2,
            "comb": np.ascontiguousarray(comb[c * NW:(c + 1) * NW]),
        })
    return in_maps, newid


def _build_program(ncores):
    import concourse.bass as bass
    import concourse.bass_isa as bass_isa
    import concourse.tile as tile
    from concourse import bacc, mybir
    from contextlib import ExitStack

    f32 = mybir.dt.float32
    bf16 = mybir.dt.bfloat16
    f16 = mybir.dt.float16
    i16 = mybir.dt.int16

    nc = bacc.Bacc(
        "TRN2", target_bir_lowering=False, debug=False, num_devices=ncores
    )

    ftT_in = nc.dram_tensor("ftT", [128, 2, SH], f32, kind="ExternalInput").ap()
    Wt4_in = nc.dram_tensor("Wt4", [128, 2, 2, 128], f32, kind="ExternalInput").ap()
    a4_in = nc.dram_tensor("attn4", [128, 2, 2], f32, kind="ExternalInput").ap()
    b2_in = nc.dram_tensor("bias2", [128, 2], f32, kind="ExternalInput").ap()
    comb_in = nc.dram_tensor("comb", [NW, 128, 544], i16, kind="ExternalInput").ap()
    out_ext = nc.dram_tensor("out", [128, 2, SH], f32, kind="ExternalOutput").ap()

    hstage = nc.dram_tensor("hstage", [128, SH * 4], bf16).ap()
    if ncores > 1:
        hfull = nc.dram_tensor("hfull", [ncores, 128, SH * 4], bf16,
                               addr_space="Shared").ap()
    else:
        hfull = hstage

    add = mybir.AluOpType.add
    mult = mybir.AluOpType.mult
    amax = mybir.AluOpType.max
    sub = mybir.AluOpType.subtract

    with tile.TileContext(nc) as tc, ExitStack() as ctx:
        const = ctx.enter_context(tc.tile_pool(name="const", bufs=1))
        wt = const.tile([128, 2, 2, 128], f32, tag="wt")
        nc.sync.dma_start(wt[:], Wt4_in[:])
        a4 = const.tile([128, 2, 2], f32, tag="a4")
        nc.sync.dma_start(a4[:], a4_in[:])
        b2 = const.tile([128, 2], f32, tag="b2")
        nc.sync.dma_start(b2[:], b2_in[:])
        combs = const.tile([128, NW, 544], i16, tag="combs")
        nc.sync.dma_start(combs[:], comb_in.transpose([1, 0, 2]))
        zrow = const.tile([128, 1], f32, tag="zrow")
        nc.vector.memset(zrow[:], 0.0)
        usall = const.tile([128, 3, SH], f32, tag="usall")
        er_rep = const.tile([128, SH], f32, tag="er_rep")

        for _rep in range(REPEAT):
            # ---- Phase 1: feature-major h + el/er for the local shard ----
            with tc.tile_pool(name="p1", bufs=1) as p1:
                ftT = p1.tile([128, 2, SH], f32, tag="ftT")
                nc.sync.dma_start(ftT[:], ftT_in[:])

                bounds = [(0, 512), (512, 1024), (1024, 1280)]
                hst = p1.tile([128, SH, 4], bf16, tag="hst")
                t4 = p1.tile([128, 2, 2, SH], f32, tag="t4")
                with tc.tile_pool(name="p1psh", bufs=1, space="PSUM") as pph:
                    psh = pph.tile([128, 2, 1536], f32, tag="psh")
                    for ft in range(2):
                        for kh in range(2):
                            for c0, c1 in bounds:
                                nc.tensor.matmul(
                                    psh[:, ft, c0:c1], lhsT=wt[:, kh, ft, :],
                                    rhs=ftT[:, kh, c0:c1],
                                    start=(kh == 0), stop=(kh == 1))
                    nc.vector.tensor_copy(hst[:, :, 0], psh[:, 0, 0:SH])
                    nc.vector.tensor_copy(hst[:, :, 1], psh[:, 1, 0:SH])
                    # t4[p, lr, ft, n] = attn4[p, lr, ft] * hT[p, ft, n];
                    # partition-sum + lane-pair add = el/er, replicated
                    nc.vector.tensor_tensor(
                        out=t4[:],
                        in0=psh[:, None, :, 0:SH].broadcast_to([128, 2, 2, SH]),
                        in1=a4[:, :, :, None].broadcast_to([128, 2, 2, SH]),
                        op=mult)
                t4r = p1.tile([128, 2, 2, SH], f32, tag="t4r")
                nc.gpsimd.partition_all_reduce(
                    t4r[:].rearrange("p a b n -> p (a b n)"),
                    t4[:].rearrange("p a b n -> p (a b n)"),
                    channels=128, reduce_op=bass_isa.ReduceOp.add)
                el_rep = p1.tile([128, SH], f32, tag="el_rep")
                nc.vector.tensor_tensor(out=el_rep[:], in0=t4r[:, 0, 0, :],
                                        in1=t4r[:, 0, 1, :], op=add)
                nc.vector.tensor_tensor(out=er_rep[:], in0=t4r[:, 1, 0, :],
                                        in1=t4r[:, 1, 1, :], op=add)
                # el as fp16 bits in the bf16 lane 2 (abs err ~6e-4);
                # lane 3 is a pad the gather fetches but nothing reads
                nc.vector.tensor_copy(hst[:, :, 2].bitcast(f16), el_rep[:])
                nc.sync.dma_start(
                    hstage[:], hst[:].rearrange("p n d -> p (n d)"))

            if ncores > 1:
                nc.gpsimd.collective_compute(
                    "AllGather", mybir.AluOpType.bypass,
                    replica_groups=[list(range(ncores))],
                    ins=[hstage[:]], outs=[hfull[:]],
                )

            # ---- Phase 2: per-window gather + softmax + prefix-sum agg ----
            with tc.tile_pool(name="p2", bufs=1) as p2, \
                 tc.tile_pool(name="p2ps", bufs=1, space="PSUM") as pp2:
                hTi = p2.tile([128, ncores * SH * 4], bf16, tag="hTi")
                if ncores > 1:
                    nc.sync.dma_start(hTi[:], hfull.transpose([1, 0, 2]))
                else:
                    nc.sync.dma_start(hTi[:], hfull[:])
                hTv = hTi[:].rearrange("p (n d) -> p n d", d=4)

                wh = p2.tile([128, 3, CW], bf16, tag="wh")
                nc.vector.memset(wh[:, :, 0:4], 0.0)
                slot = p2.tile([128, 3 * CW], f32, tag="slot")
                erg = p2.tile([128, EPW], f32, tag="erg")
                bnd = p2.tile([128, 400], f32, tag="bnd")
                negM = p2.tile([128, 1], f32, tag="negM")
                pe = pp2.tile([128, EPW], f32, tag="pe")

                ghr = slot[:, 0:2 * EPW].bitcast(bf16).rearrange(
                    "p (e d) -> p e d", d=4)               # [128, 4096, 4]
                e2v = slot[:, 2 * EPW:3 * EPW]             # [128, 4096] f32

                for w in range(NW):
                    hix = combs[:, w, 0:256]
                    eix = combs[:, w, 256:512]
                    bix = combs[:, w, 512:537]
                    nc.gpsimd.ap_gather(ghr, hTv, hix, channels=128,
                                        num_elems=ncores * SH, d=4,
                                        num_idxs=EPW)
                    nc.gpsimd.ap_gather(erg[:, :, None],
                                        er_rep[:, :, None], eix,
                                        channels=128, num_elems=SH, d=1,
                                        num_idxs=EPW)
                    # e = el + er  (el was stored as fp16 bits)
                    nc.vector.tensor_tensor(out=e2v,
                                            in0=ghr[:, :, 2].bitcast(f16),
                                            in1=erg[:], op=add)
                    # e = leakyrelu(e) = max(0.2*e, e)
                    nc.vector.scalar_tensor_tensor(
                        out=pe[:], in0=e2v, scalar=NEG_SLOPE, in1=e2v,
                        op0=mult, op1=amax)
                    # per-window max (negated) for a safe exp
                    nc.vector.tensor_reduce(
                        out=negM[:], in_=pe[:], axis=mybir.AxisListType.X,
                        op=amax, negate=True)
                    # w = exp(e - M) -> channel 2 of the scan buffer
                    nc.scalar.activation(
                        wh[:, 2, 4:4 + EPW], pe[:],
                        mybir.ActivationFunctionType.Exp,
                        bias=negM[:, 0:1])
                    # wh channels 0,1 = w * h-pair
                    nc.vector.tensor_tensor(
                        out=wh[:, 0:2, 4:4 + EPW].transpose([0, 2, 1]),
                        in0=ghr[:, :, 0:2],
                        in1=wh[:, 2, 4:4 + EPW][:, :, None].broadcast_to(
                            [128, EPW, 2]),
                        op=mult)
                    # one prefix scan over all 3 channels
                    nc.vector.tensor_tensor_scan(
                        out=slot[:],
                        data0=wh[:].rearrange("p a b -> p (a b)"),
                        data1=zrow[:].broadcast_to([128, 3 * CW]),
                        initial=0.0, op0=add, op1=add)
                    # segment boundary prefix values, then diff -> (u0,u1,s)
                    nc.gpsimd.ap_gather(bnd[:, :, None], slot[:, :, None],
                                        bix, channels=128,
                                        num_elems=3 * CW, d=1, num_idxs=400)
                    bv = bnd[:, 0:396].rearrange("p (k j) -> p k j", k=3)
                    nc.vector.tensor_tensor(
                        out=usall[:, :, w * 128:(w + 1) * 128],
                        in0=bv[:, :, 1:129], in1=bv[:, :, 0:128], op=sub)

            # ---- Epilogue: out^T = u / s + bias ----
            with tc.tile_pool(name="ep", bufs=1) as ep:
                rcp = ep.tile([128, SH], f32, tag="rcp")
                nc.vector.reciprocal(rcp[:], usall[:, 2, :])
                ot = ep.tile([128, 2, SH], f32, tag="ot")
                nc.vector.tensor_tensor(
                    out=ot[:], in0=usall[:, 0:2, :],
                    in1=rcp[:, None, :].broadcast_to([128, 2, SH]), op=mult)
                ot2 = ep.tile([128, 2, SH], f32, tag="ot2")
                nc.vector.tensor_tensor(
                    out=ot2[:], in0=ot[:],
                    in1=b2[:, :, None].broadcast_to([128, 2, SH]), op=add)
                nc.sync.dma_start(out_ext[:], ot2[:])

    nc.compile()
    return nc


def _get_program(ncores):
    if ncores not in _prog_cache:
        _prog_cache[ncores] = _build_program(ncores)
    return _prog_cache[ncores]


def kernel(feats, W, attn_l, attn_r, bias, src, dst):
    from concourse.bass_utils import run_bass_kernel_spmd

    feats = np.asarray(feats, np.float32)
    W = np.asarray(W, np.float32)
    attn_l = np.asarray(attn_l, np.float32)
    attn_r = np.asarray(attn_r, np.float32)
    bias = np.asarray(bias, np.float32)
    src = np.asarray(src)
    dst = np.asarray(dst)

    in_maps, newid = _prep_inputs(feats, W, attn_l, attn_r, bias, src, dst)
    nc = _get_program(NCORES)
    res = run_bass_kernel_spmd(nc, in_maps, list(range(NCORES)))
    shards = []
    for c in range(NCORES):
        o = np.asarray(res.results[c]["out"])          # [128, 2, SH]
        shards.append(o.transpose(2, 1, 0).reshape(SH, D))
    out_cat = np.concatenate(shards, axis=0)
    return out_cat[newid[:N]].astype(np.float32)
